# revision 24
# baseline (speedup 1.0000x reference)
"""DGL JT-NN decoder forward on 8 Trainium2 NeuronCores (Bass/Tile).

Data-parallel over the B (tree) axis: each of the 8 cores processes 256 trees.
Weights are replicated. Each core returns 4 partial sums
(q_loss_sum, q_correct_cnt, p_bce_sum, p_sign_partial); the host combines them.

v2 layout (per core, B=256 local trees):
  - The fwd and rev GRU scans are MERGED along the free dim: state tiles are
    [128 part (feature blk), 2 (blk), 512] where cols 0:256 = fwd batch and
    cols 256:512 = rev batch, all float32r. This halves scan instruction
    count and doubles per-matmul work vs two separate 256-wide scans.
  - One ap_gather per step fetches x[t]|x[23-t] for both feature blocks
    (1024 idxs from a [128, 2V] emb.T table, rev/blk offsets precomputed).
  - GRU biases are applied via the ACT bias operand (per-j-block), not
    bias-row matmuls.
  - p head: down group t+1 runs 256-wide; the two up groups that become
    ready at step t>=11 pair into one 512-wide unit whose x/m operands are
    exactly the merged tiles; per-unit p sums go psum->sbuf->DRAM with one
    packed load-back.
  - q head: hidden layers computed inside the scan (relu is in every ACT
    table set); the 780-wide exp/logit work runs post-scan so the ACT table
    set switches only twice per rep (sigmoid/tanh set <-> exp/ln set).
"""
import numpy as np
from contextlib import ExitStack

import concourse.bass as bass
import concourse.tile as tile
from concourse import bacc, mybir
from concourse.tile_rust import add_dep_helper

import jax
from jax.sharding import Mesh, PartitionSpec
from jax.experimental.shard_map import shard_map
from concourse.bass2jax import install_neuronx_cc_hook, _bass_exec_p, partition_id_tensor

dt = mybir.dt
F32, F32R, BF16, I16 = dt.float32, dt.float32r, dt.bfloat16, dt.int16
AF = mybir.ActivationFunctionType
OP = mybir.AluOpType

P = 128
B_FULL, D, H, L, V = 2048, 24, 256, 64, 780
NCORES = 8
B = B_FULL // NCORES          # 256 trees per core
B2 = 2 * B                    # merged fwd|rev batch
HB = H // P                   # 2 feature blocks
NQ = D                        # q groups (root + 23 down)
NQB = NQ * (B // P)           # 48 q row-blocks
NP = 2 * D - 1                # 47 p groups
NPROWS = NP * B               # 12032 p rows per core
PF = NPROWS // P              # 94 packed p columns
VC = 390                      # logits free-dim chunk (2 chunks of 390)

LAST_SCAN_ACT = [None]
TAIL_ACTS = []


def build_nc(reps=1):
    nc = bacc.Bacc(None, target_bir_lowering=False)

    # ---- DRAM I/O ----  (f32r tensors receive plain f32 host arrays)
    d_wz = nc.dram_tensor("wz", [P, 4, H], F32R, kind="ExternalInput")
    d_wh = nc.dram_tensor("wh", [P, 4, H], F32R, kind="ExternalInput")
    d_wr = nc.dram_tensor("wr", [P, 2, H], F32R, kind="ExternalInput")
    d_ur = nc.dram_tensor("ur", [P, 2, H], F32R, kind="ExternalInput")
    d_ww = nc.dram_tensor("ww", [P, 3, H], F32R, kind="ExternalInput")
    d_uw = nc.dram_tensor("uw", [P, 5, H], F32R, kind="ExternalInput")
    d_wo = nc.dram_tensor("wo", [P, 2, V], F32R, kind="ExternalInput")
    d_wob = nc.dram_tensor("wob", [1, V], F32R, kind="ExternalInput")
    d_us = nc.dram_tensor("us", [P, 2, 1], F32R, kind="ExternalInput")
    d_gb = nc.dram_tensor("gbias", [P, 6], F32, kind="ExternalInput")
    d_emb = nc.dram_tensor("embt", [P, 2 * V], F32, kind="ExternalInput")
    d_tvq = nc.dram_tensor("tvq2", [P, B2], F32R, kind="ExternalInput")
    d_xidx = nc.dram_tensor("xidx", [P, D * 64], I16, kind="ExternalInput")
    d_qtidx = nc.dram_tensor("qtidx", [P, NQB], I16, kind="ExternalInput")
    d_eye16 = nc.dram_tensor("eye16", [P, 16], F32, kind="ExternalInput")
    d_ptm = nc.dram_tensor("ptm", [P, PF], F32, kind="ExternalInput")
    d_ptneg = nc.dram_tensor("ptneg", [P, PF], F32, kind="ExternalInput")
    d_usb = nc.dram_tensor("usb", [P, 1], F32, kind="ExternalInput")
    d_out = nc.dram_tensor("out", [1, 4], F32, kind="ExternalOutput")

    with tile.TileContext(nc) as tc, ExitStack() as ctx:
        const = ctx.enter_context(tc.tile_pool(name="const", bufs=1))
        xw = ctx.enter_context(tc.tile_pool(name="xw", bufs=3))
        xwf = ctx.enter_context(tc.tile_pool(name="xwf", bufs=2))
        mlong = ctx.enter_context(tc.tile_pool(name="mlong", bufs=11))
        mshort = ctx.enter_context(tc.tile_pool(name="mshort", bufs=3))
        hidp = ctx.enter_context(tc.tile_pool(name="hidp", bufs=NQ))
        trans = ctx.enter_context(tc.tile_pool(name="trans", bufs=2))
        psA = ctx.enter_context(tc.tile_pool(name="psA", bufs=2, space="PSUM"))
        psB = ctx.enter_context(tc.tile_pool(name="psB", bufs=2, space="PSUM"))
        dramp = ctx.enter_context(tc.tile_pool(name="dramp", bufs=1, space="DRAM"))

        w = {}

        def load_direct(dram, shape, name, cdt=F32R):
            t = const.tile(shape, cdt, name=name)
            nc.sync.dma_start(t[:], dram.ap())
            return t

        w["wz"] = load_direct(d_wz, [P, 4, H], "wz")
        w["wh"] = load_direct(d_wh, [P, 4, H], "wh")
        w["wr"] = load_direct(d_wr, [P, 2, H], "wr")
        w["ur"] = load_direct(d_ur, [P, 2, H], "ur")
        w["ww"] = load_direct(d_ww, [P, 3, H], "ww")
        w["uw"] = load_direct(d_uw, [P, 5, H], "uw")
        w["wo"] = load_direct(d_wo, [P, 2, V], "wo")
        w["wob"] = load_direct(d_wob, [1, V], "wob")
        w["us"] = load_direct(d_us, [P, 2, 1], "us")
        w["emb"] = load_direct(d_emb, [P, 2 * V], "emb", cdt=F32)
        w["tvq"] = load_direct(d_tvq, [P, B2], "tvq")
        gb = const.tile([P, 6], F32, name="gbias")
        nc.sync.dma_start(gb[:], d_gb.ap())
        ones_f = const.tile([1, P], F32, name="ones_f")
        nc.any.memset(ones_f[:], 1.0)
        ones_row = const.tile([1, P], F32R, name="ones_row")
        nc.vector.tensor_copy(ones_row[:], ones_f[:])
        onescol = const.tile([P, 1], F32, name="onescol")
        nc.any.memset(onescol[:], 1.0)
        eye16 = const.tile([P, 16], F32, name="eye16")
        nc.sync.dma_start(eye16[:], d_eye16.ap())
        ptm = const.tile([P, PF], F32, name="ptm")
        nc.sync.dma_start(ptm[:], d_ptm.ap())
        ptneg = const.tile([P, PF], F32, name="ptneg")
        nc.sync.dma_start(ptneg[:], d_ptneg.ap())
        usb = const.tile([P, 1], F32, name="usb")
        nc.sync.dma_start(usb[:], d_usb.ap())
        xidx = const.tile([P, D * 64], I16, name="xidx")
        nc.sync.dma_start(xidx[:], d_xidx.ap())
        qtidx = const.tile([P, NQB], I16, name="qtidx")
        nc.sync.dma_start(qtidx[:], d_qtidx.ap())

        wz, wh, wr, ur, ww, uw = (w[k] for k in ("wz", "wh", "wr", "ur", "ww", "uw"))
        tvq = w["tvq"]

        loop_cm = tc.For_i(0, reps, 1) if reps > 1 else None
        if loop_cm is not None:
            loop_cm.__enter__()

        # per-rep buffers
        selbuf = trans.tile([P, 2 * NQB], F32, name="selbuf", tag="selbuf", bufs=1)
        cntbuf = trans.tile([P, NQB], F32, name="cntbuf", tag="cntbuf", bufs=1)
        p_dram = dramp.tile([1, NPROWS], F32, name="p_dram", tag="p_dram")

        def gather_x(k):
            """stage 1: gather mx(k) = [x[k] | x[23-k]] into an f32 tile.
            (The gpsimd ucode only knows plain dtypes, and walrus requires
            fp32r matmul operands to come from an f32r-rounding producer, so
            the cast to f32r is a separate DVE copy -- see cast_x.)"""
            xs = xwf.tile([P, HB, B2], F32, name="mxf", tag="mxf", bufs=2)
            nc.gpsimd.ap_gather(xs[:].rearrange("p a b -> p (a b)"),
                                w["emb"][:], xidx[:, k * 64:(k + 1) * 64],
                                channels=P, num_elems=2 * V, d=1, num_idxs=4 * B)
            return xs

        def cast_x(xs_f):
            xs = xw.tile([P, HB, B2], F32R, name="mx", tag="mx")
            nc.vector.tensor_copy(xs[:], xs_f[:])
            return xs

        def p_x_tv(ps, x, xsl, width, close=False):
            """open the p-unit psum group: x + tv matmuls."""
            for j in range(HB):
                for kb in range(HB):
                    nc.tensor.matmul(ps[:, j, 0:width], uw[:, kb, bass.ts(j, P)],
                                     x[:, kb, xsl], start=(kb == 0), stop=False)
                nc.tensor.matmul(ps[:, j, 0:width], uw[:, 4, bass.ts(j, P)],
                                 tvq[:, 0:width], start=False, stop=close)

        def p_h_close(ps, m_fresh, m_old, width):
            """h matmuls + close group (x/tv already queued full-width)."""
            assert m_old is None
            for j in range(HB):
                for kb in range(HB):
                    nc.tensor.matmul(ps[:, j, 0:width], uw[:, 2 + kb, bass.ts(j, P)],
                                     m_fresh[:, kb, 0:width], start=False,
                                     stop=(kb == HB - 1))

        def p_up_pair(ps, x, m_fresh, m_old):
            """up-pair with cross terms. PSUM groups are bank-granular with a
            single exact region, so each half runs as its own complete group,
            sequentially within each j bank.
            half A (cols 0:B)  = up(21-t): x fwd, m_fresh fwd, m_old rev.
            half B (cols B:B2) = up(t):    x rev, m_fresh rev, m_old fwd."""
            for j in range(HB):
                for half in range(2):
                    sl = slice(half * B, (half + 1) * B)
                    osl = slice((1 - half) * B, (2 - half) * B)
                    for kb in range(HB):
                        nc.tensor.matmul(ps[:, j, sl], uw[:, kb, bass.ts(j, P)],
                                         x[:, kb, sl], start=(kb == 0), stop=False)
                    nc.tensor.matmul(ps[:, j, sl], uw[:, 4, bass.ts(j, P)],
                                     tvq[:, 0:B], start=False, stop=False)
                    for kb in range(HB):
                        nc.tensor.matmul(ps[:, j, sl], uw[:, 2 + kb, bass.ts(j, P)],
                                         m_old[:, kb, osl], start=False, stop=False)
                    for kb in range(HB):
                        nc.tensor.matmul(ps[:, j, sl], uw[:, 2 + kb, bass.ts(j, P)],
                                         m_fresh[:, kb, sl], start=False,
                                         stop=(kb == HB - 1))

        def p_finish(ps, width, off, tag):
            """relu -> us matvec -> copy to sbuf -> DMA to p_dram[off:off+width]."""
            ph = trans.tile([P, HB, width], F32R, name=f"ph{tag}", tag=f"ph{tag}",
                            bufs=1)
            nc.scalar.activation(ph[:], ps[:, :, 0:width], AF.Relu)
            ps_s = psB.tile([1, width], F32, name="ps_pv", tag="psB",
                            padded_shape=[1, 1024])
            for kb in range(HB):
                nc.tensor.matmul(ps_s[:, :], w["us"][:, kb, 0:1], ph[:, kb, :],
                                 start=(kb == 0), stop=(kb == HB - 1))
            pv = trans.tile([1, width], F32, name=f"pv{tag}", tag=f"pv{tag}", bufs=1)
            nc.scalar.copy(pv[:], ps_s[:, :])
            nc.sync.dma_start(p_dram[0:1, bass.ds(off, width)], pv[:])

        def q_hid(g, m_prev_tile):
            ps_h = psB.tile([P, HB, B], F32, name="ps_qh", tag="psB",
                            padded_shape=[P, HB, 512])
            for j in range(HB):
                first = True
                if m_prev_tile is not None:
                    for kb in range(HB):
                        nc.tensor.matmul(ps_h[:, j, :], ww[:, kb, bass.ts(j, P)],
                                         m_prev_tile[:, kb, 0:B],
                                         start=first, stop=False)
                        first = False
                nc.tensor.matmul(ps_h[:, j, :], ww[:, 2, bass.ts(j, P)],
                                 tvq[:, 0:B], start=first, stop=True)
            hid = hidp.tile([P, HB, B], F32R, name="qhid", tag="qhid", bufs=NQ)
            nc.scalar.activation(hid[:], ps_h[:], AF.Relu)
            return hid

        # ---------------- prologue ----------------
        mx = {0: cast_x(gather_x(0)), 1: cast_x(gather_x(1))}
        hid_list = {0: q_hid(0, None)}
        # root p unit: x[0], h=0, tv  -> slot 0
        ps_root = psB.tile([P, HB, B], F32, name="ps_phD", tag="psB",
                           padded_shape=[P, HB, 512])
        p_x_tv(ps_root, mx[0], slice(0, B), B, close=True)
        p_finish(ps_root, B, 0, "D")
        # step-0 zmt x-part matmuls (no m parts at t=0: groups close here)
        ps_z = psA.tile([P, HB, B2], F32, name="ps_z", tag="psA")
        ps_m = psA.tile([P, HB, B2], F32, name="ps_m", tag="psA")
        for j in range(HB):
            for kb in range(HB):
                nc.tensor.matmul(ps_z[:, j, :], wz[:, kb, bass.ts(j, P)],
                                 mx[0][:, kb, :], start=(kb == 0), stop=(kb == 1))
            for kb in range(HB):
                nc.tensor.matmul(ps_m[:, j, :], wh[:, kb, bass.ts(j, P)],
                                 mx[0][:, kb, :], start=(kb == 0), stop=(kb == 1))

        # ---------------- main scan loop ----------------
        m = {}
        m_prev = rm_prev = None
        for t in range(D - 1):
            last = (t == D - 2)
            # (a) prefetch gather (cast to f32r happens after the combine)
            gf = gather_x(t + 2) if t + 2 <= D - 1 else None
            # (b) m-part matmuls for step t (x parts queued last iteration)
            if t > 0:
                for j in range(HB):
                    for kb in range(HB):
                        nc.tensor.matmul(ps_z[:, j, :], wz[:, 2 + kb, bass.ts(j, P)],
                                         m_prev[:, kb, :], start=False,
                                         stop=(kb == 1))
                    for kb in range(HB):
                        nc.tensor.matmul(ps_m[:, j, :], wh[:, 2 + kb, bass.ts(j, P)],
                                         rm_prev[:, kb, :], start=False,
                                         stop=(kb == 1))
            # (c) gate activations (bias via ACT operand, per j block)
            z = trans.tile([P, HB, B2], F32R, name="z", tag="z", bufs=1)
            mt = trans.tile([P, HB, B2], F32R, name="mt", tag="mt", bufs=1)
            for j in range(HB):
                nc.scalar.activation(z[:, j, :], ps_z[:, j, :], AF.Sigmoid,
                                     bias=gb[:, j:j + 1])
            for j in range(HB):
                nc.scalar.activation(mt[:, j, :], ps_m[:, j, :], AF.Tanh,
                                     bias=gb[:, 4 + j:5 + j])
            # (d) combine -> m(t)
            mpool = mlong if t <= 10 else mshort
            m_t = mpool.tile([P, HB, B2], F32R, name="m", tag="m")
            if t == 0:
                nc.vector.tensor_tensor(m_t[:], z[:], mt[:], op=OP.mult)
            else:
                t1 = trans.tile([P, HB, B2], F32R, name="t1", tag="t1", bufs=1)
                nc.vector.tensor_tensor(t1[:], mt[:], m_prev[:], op=OP.subtract)
                nc.vector.tensor_tensor(t1[:], t1[:], z[:], op=OP.mult)
                nc.vector.tensor_tensor(m_t[:], m_prev[:], t1[:], op=OP.add)
            m[t] = m_t
            # (e) PE fill during the ACT/DVE chain: r x-part, q hid, p x/tv
            ps_r = psA.tile([P, HB, B2], F32, name="ps_r", tag="psA")
            for j in range(HB):
                for kb in range(HB):
                    nc.tensor.matmul(ps_r[:, j, :], wr[:, kb, bass.ts(j, P)],
                                     mx[t + 1][:, kb, :], start=(kb == 0),
                                     stop=False)
            if t > 0:
                hid_list[t] = q_hid(t, m[t - 1])
            ps_hD = ps_hU = None
            if t <= 21:
                ps_hD = psB.tile([P, HB, B], F32, name="ps_phD", tag="psB",
                                 padded_shape=[P, HB, 512])
                p_x_tv(ps_hD, mx[t + 1], slice(0, B), B)
            if t == 22:
                ps_hU = psB.tile([P, HB, B2], F32, name="ps_phU", tag="psB")
                p_x_tv(ps_hU, mx[t + 1], slice(0, B2), B2)
            # (g) r m-part matmuls (wait on m(t))
            for j in range(HB):
                for kb in range(HB):
                    nc.tensor.matmul(ps_r[:, j, :], ur[:, kb, bass.ts(j, P)],
                                     m_t[:, kb, :], start=False, stop=(kb == 1))
            # (f) next-step zmt x-part matmuls
            if not last:
                ps_z = psA.tile([P, HB, B2], F32, name="ps_z", tag="psA")
                ps_m = psA.tile([P, HB, B2], F32, name="ps_m", tag="psA")
                for j in range(HB):
                    for kb in range(HB):
                        nc.tensor.matmul(ps_z[:, j, :], wz[:, kb, bass.ts(j, P)],
                                         mx[t + 1][:, kb, :], start=(kb == 0),
                                         stop=False)
                    for kb in range(HB):
                        nc.tensor.matmul(ps_m[:, j, :], wh[:, kb, bass.ts(j, P)],
                                         mx[t + 1][:, kb, :], start=(kb == 0),
                                         stop=False)
            # (h) r activation + rm
            r = trans.tile([P, HB, B2], F32R, name="r", tag="r", bufs=1)
            for j in range(HB):
                a = nc.scalar.activation(r[:, j, :], ps_r[:, j, :], AF.Sigmoid,
                                         bias=gb[:, 2 + j:3 + j])
            LAST_SCAN_ACT[0] = a
            rm_t = trans.tile([P, HB, B2], F32R, name="rm", tag="rm", bufs=1)
            nc.vector.tensor_tensor(rm_t[:], r[:], m_t[:], op=OP.mult)
            if gf is not None:
                mx[t + 2] = cast_x(gf)
            # (i) p-unit h matmuls + close groups
            if ps_hD is not None:
                p_h_close(ps_hD, m_t, None, B)
            if 11 <= t <= 21:
                ps_hU = psB.tile([P, HB, B2], F32, name="ps_phU", tag="psB")
                p_up_pair(ps_hU, mx[t + 1], m_t, m[21 - t])
            elif ps_hU is not None:
                p_h_close(ps_hU, m_t, None, B2)
            # (j) p-unit tails
            if ps_hD is not None:
                p_finish(ps_hD, B, B + t * B, "D")
            if ps_hU is not None:
                off = (23 * B + (t - 11) * B2) if t <= 21 else (23 * B + 11 * B2)
                p_finish(ps_hU, B2, off, "U")
            m_prev, rm_prev = m_t, rm_t

        hid_list[D - 1] = q_hid(D - 1, m[D - 2])

        # ---------------- q phase (exp/ln table set) ----------------
        for g in range(NQ):
            hid = hid_list[g]
            for rb in range(B // P):
                col = g * (B // P) + rb
                ps_l = psB.tile([P, 2, 512], F32, name="ps_l", tag="psB")
                for c in range(2):
                    for kb in range(HB):
                        nc.tensor.matmul(ps_l[:, c, :VC],
                                         hid[:, kb, bass.ts(rb, P)],
                                         w["wo"][:, kb, bass.ds(c * VC, VC)],
                                         start=(kb == 0), stop=False)
                    nc.tensor.matmul(ps_l[:, c, :VC], ones_row[0:1, 0:P],
                                     w["wob"][0:1, bass.ds(c * VC, VC)],
                                     start=False, stop=True)
                exp_t = trans.tile([P, V], F32, name="exp_t", tag="exp_t", bufs=1)
                TAIL_ACTS.append(nc.scalar.activation(
                    exp_t[:].rearrange("p (c v) -> p c v", c=2), ps_l[:, :, :VC],
                    AF.Exp, accum_out=selbuf[:, NQB + col:NQB + col + 1]))
                g16 = trans.tile([P, 16], F32, name="g16", tag="g16", bufs=2)
                nc.gpsimd.ap_gather(g16[:], exp_t[:], qtidx[:, col:col + 1],
                                    channels=P, num_elems=V, d=1, num_idxs=16)
                junk16 = trans.tile([P, 16], F32, name="junk16", tag="junk16",
                                    bufs=2)
                nc.vector.scalar_tensor_tensor(
                    junk16[:], g16[:], 1.0, eye16[:], op0=OP.mult, op1=OP.mult,
                    accum_out=selbuf[:, col:col + 1])
                junkv = trans.tile([P, V], F32, name="junkv", tag="junkv", bufs=1)
                nc.vector.tensor_scalar(
                    junkv[:], exp_t[:], selbuf[:, col:col + 1], None,
                    op0=OP.is_gt, op1=OP.add, accum_out=cntbuf[:, col:col + 1])

        # ---------------- p losses ----------------
        p_pack = trans.tile([P, PF], F32, name="p_pack", tag="p_pack", bufs=1)
        nc.sync.dma_start(p_pack[:],
                          p_dram[:].rearrange("o (p f) -> (o p) f", p=P))
        nc.vector.tensor_scalar(p_pack[:], p_pack[:], usb[:, 0:1], None, op0=OP.add)
        redbuf = trans.tile([P, 4], F32, name="redbuf", tag="redbuf", bufs=1)
        t_relu = trans.tile([P, PF], F32, name="t_relu", tag="t_relu", bufs=1)
        nc.vector.tensor_scalar(t_relu[:], p_pack[:], 0.0, None, op0=OP.max)
        t_pt = trans.tile([P, PF], F32, name="t_pt", tag="t_pt", bufs=1)
        nc.vector.tensor_tensor(t_pt[:], p_pack[:], ptm[:], op=OP.mult)
        t_abs = trans.tile([P, PF], F32, name="t_abs", tag="t_abs", bufs=1)
        nc.vector.scalar_tensor_tensor(t_abs[:], p_pack[:], -1.0, p_pack[:],
                                       op0=OP.mult, op1=OP.max)
        t_en = trans.tile([P, PF], F32, name="t_en", tag="t_en", bufs=1)
        TAIL_ACTS.append(nc.scalar.activation(t_en[:], t_abs[:], AF.Exp, scale=-1.0))
        t_l1p = trans.tile([P, PF], F32, name="t_l1p", tag="t_l1p", bufs=1)
        TAIL_ACTS.append(nc.scalar.activation(t_l1p[:], t_en[:], AF.Ln, bias=1.0))
        nc.vector.tensor_tensor(t_relu[:], t_relu[:], t_pt[:], op=OP.subtract)
        nc.vector.tensor_tensor(t_relu[:], t_relu[:], t_l1p[:], op=OP.add)
        nc.vector.reduce_sum(redbuf[:, 2:3], t_relu[:], axis=mybir.AxisListType.X)
        pmask = trans.tile([P, PF], F32, name="pmask", tag="pmask", bufs=1)
        nc.vector.tensor_scalar(pmask[:], p_pack[:], 0.0, None, op0=OP.is_gt)
        junkp = trans.tile([P, PF], F32, name="junkp", tag="junkp", bufs=1)
        nc.vector.scalar_tensor_tensor(junkp[:], pmask[:], 1.0, ptneg[:],
                                       op0=OP.mult, op1=OP.mult,
                                       accum_out=redbuf[:, 3:4])

        # ---------------- q losses ----------------
        loged = trans.tile([P, 2 * NQB], F32, name="loged", tag="loged", bufs=1)
        TAIL_ACTS.append(nc.scalar.activation(loged[:], selbuf[:], AF.Ln))
        qdiff = trans.tile([P, NQB], F32, name="qdiff", tag="qdiff", bufs=1)
        nc.vector.tensor_tensor(qdiff[:], loged[:, NQB:], loged[:, :NQB],
                                op=OP.subtract)
        nc.vector.reduce_sum(redbuf[:, 0:1], qdiff[:], axis=mybir.AxisListType.X)
        junkc = trans.tile([P, NQB], F32, name="junkc", tag="junkc", bufs=1)
        nc.vector.tensor_scalar(junkc[:], cntbuf[:], 0.0, None,
                                op0=OP.is_equal, op1=OP.add,
                                accum_out=redbuf[:, 1:2])

        # ---------------- final cross-partition reduce ----------------
        ps_f = psB.tile([1, 4], F32, name="ps_f", tag="psB", padded_shape=[1, 1024])
        nc.tensor.matmul(ps_f[:, :], onescol[:], redbuf[:], start=True, stop=True)
        outt = trans.tile([1, 4], F32, name="outt", tag="outt", bufs=1)
        nc.scalar.copy(outt[:], ps_f[:, :])
        nc.sync.dma_start(d_out.ap(), outt[:])

        # keep ACT table switches to two per rep: every Exp/Ln ACTIVATE is
        # ordered after the last scan Sigmoid.
        if LAST_SCAN_ACT[0] is not None:
            for inst in TAIL_ACTS:
                add_dep_helper(inst.ins, LAST_SCAN_ACT[0].ins, sync=False,
                               reason="ACT table phase ordering")
        TAIL_ACTS.clear()
        LAST_SCAN_ACT[0] = None

        if loop_cm is not None:
            loop_cm.__exit__(None, None, None)
    nc.compile()
    return nc


# ---------------- host side ----------------

_RUNNER = {}


class _BassRunner:
    def __init__(self, nc, n_cores):
        install_neuronx_cc_hook()
        self.nc = nc
        self.n_cores = n_cores
        partition_name = nc.partition_id_tensor.name if nc.partition_id_tensor else None
        in_names, out_names, out_avals, zero_outs = [], [], [], []
        for alloc in nc.m.functions[0].allocations:
            if not isinstance(alloc, mybir.MemoryLocationSet):
                continue
            name = alloc.memorylocations[0].name
            if alloc.kind == "ExternalInput":
                if name != partition_name:
                    in_names.append(name)
            elif alloc.kind == "ExternalOutput":
                out_names.append(name)
                shape = tuple(alloc.tensor_shape)
                dtype = mybir.dt.np(alloc.dtype)
                out_avals.append(jax.core.ShapedArray(shape, dtype))
                zero_outs.append(np.zeros(shape, dtype))
        self.in_names, self.out_names = in_names, out_names
        self.out_avals, self.zero_outs = out_avals, zero_outs
        n_params, n_outs = len(in_names), len(out_names)
        self.n_params = n_params
        all_in_names = list(in_names) + list(out_names)
        if partition_name is not None:
            all_in_names.append(partition_name)

        def _body(*args):
            operands = list(args)
            if partition_name is not None:
                operands.append(partition_id_tensor())
            outs = _bass_exec_p.bind(
                *operands, out_avals=tuple(out_avals), in_names=tuple(all_in_names),
                out_names=tuple(out_names), lowering_input_output_aliases=(),
                sim_require_finite=True, sim_require_nnan=True, nc=nc)
            return tuple(outs)

        donate = tuple(range(n_params, n_params + n_outs))
        if n_cores == 1:
            self.fn = jax.jit(_body, donate_argnums=donate, keep_unused=True)
        else:
            devices = jax.devices()[:n_cores]
            mesh = Mesh(np.asarray(devices), ("core",))
            in_specs = (PartitionSpec("core"),) * (n_params + n_outs)
            out_specs = (PartitionSpec("core"),) * n_outs
            self.fn = jax.jit(
                shard_map(_body, mesh=mesh, in_specs=in_specs,
                          out_specs=out_specs, check_rep=False),
                donate_argnums=donate, keep_unused=True)

    def __call__(self, in_maps):
        n_cores = self.n_cores
        per_core = [[np.asarray(m[name]) for name in self.in_names] for m in in_maps]
        if n_cores == 1:
            args = per_core[0]
        else:
            args = [np.concatenate([per_core[c][i] for c in range(n_cores)], axis=0)
                    for i in range(self.n_params)]
        zeros = [np.zeros((n_cores * z.shape[0], *z.shape[1:]) if n_cores > 1 else z.shape,
                          z.dtype) for z in self.zero_outs]
        out_arrs = self.fn(*args, *zeros)
        jax.block_until_ready(out_arrs)
        if n_cores == 1:
            return [{name: np.asarray(out_arrs[i]) for i, name in enumerate(self.out_names)}]
        return [
            {name: np.asarray(out_arrs[i]).reshape(n_cores, *self.out_avals[i].shape)[c]
             for i, name in enumerate(self.out_names)}
            for c in range(n_cores)
        ]


def _kxm(wT):
    """[K, M] -> [128, K//128, M] K-block layout."""
    K, M = wT.shape
    assert K % P == 0
    return np.ascontiguousarray(wT.reshape(K // P, P, M).transpose(1, 0, 2))


def _prep_shared(inputs):
    f32 = np.float32
    Wz, Wh, Wr, Ur = (np.asarray(inputs[k], f32) for k in ("Wz", "Wh", "Wr", "Ur"))
    bz, br, bh = (np.asarray(inputs[k], f32) for k in ("bz", "br", "bh"))
    W_w, W_b = np.asarray(inputs["W_w"], f32), np.asarray(inputs["W_b"], f32)
    U_w, U_b = np.asarray(inputs["U_w"], f32), np.asarray(inputs["U_b"], f32)
    Wo_w, Wo_b = np.asarray(inputs["Wo_w"], f32), np.asarray(inputs["Wo_b"], f32)
    Us_w = np.asarray(inputs["Us_w"], f32)
    emb = np.asarray(inputs["emb"], f32)

    shared = {}
    shared["wz"] = _kxm(Wz.T)                      # [128, 4, 256]
    shared["wh"] = _kxm(Wh.T)
    shared["wr"] = _kxm(Wr.T)
    shared["ur"] = _kxm(Ur.T)
    wwT = np.zeros((3 * P, H), f32)
    wwT[:H] = W_w.T[:H]                            # m part
    wwT[2 * P:2 * P + L] = W_w.T[H:H + L]          # tv part
    wwT[2 * P + L] = W_b                           # bias row
    shared["ww"] = _kxm(wwT)
    uwT = np.zeros((5 * P, H), f32)
    uwT[:2 * H] = U_w.T[:2 * H]                    # x, h parts
    uwT[4 * P:4 * P + L] = U_w.T[2 * H:2 * H + L]  # tv part
    uwT[4 * P + L] = U_b                           # bias row
    shared["uw"] = _kxm(uwT)
    shared["wo"] = _kxm(Wo_w.T)                    # [128, 2, 780]
    shared["wob"] = Wo_b.reshape(1, V)
    shared["us"] = _kxm(Us_w.T)                    # [128, 2, 1]
    shared["gbias"] = np.stack(
        [bz[:P], bz[P:], br[:P], br[P:], bh[:P], bh[P:]], axis=1)   # [128, 6]
    shared["embt"] = _kxm(emb.T).reshape(P, 2 * V)  # [128, 1560]
    shared["eye16"] = np.tile(np.eye(16, dtype=f32), (8, 1))
    usb = np.asarray(inputs["Us_b"], f32).reshape(1)[0]
    shared["usb"] = np.full((P, 1), usb, f32)
    pt_rows = np.concatenate([np.ones(23 * B, f32), np.zeros(24 * B, f32)])
    ptm = pt_rows.reshape(P, PF)
    shared["ptm"] = ptm
    shared["ptneg"] = 1.0 - 2.0 * ptm
    return shared


def _wrap16(idx):
    """ap_gather index layout: idx j at [16c + j%16, j//16] for each core c."""
    n = idx.shape[0]
    arr = idx.reshape(n // 16, 16).T.astype(np.int16)   # [16, n//16]
    return np.tile(arr, (8, 1))                          # [128, n//16]


def _prep_core(inputs, c):
    f32 = np.float32
    wid = np.asarray(inputs["wid"])
    tree_vec = np.asarray(inputs["tree_vec"], f32)
    wid_loc = np.asarray(wid[c * B:(c + 1) * B], np.int64)   # [256, 24]
    tv_loc = tree_vec[c * B:(c + 1) * B]                     # [256, 64]
    per = {}
    xi = np.zeros((P, D * 64), np.int16)
    for k in range(D):
        a = wid_loc[:, k].astype(np.int64)
        b = wid_loc[:, 23 - k].astype(np.int64)
        idx = np.concatenate([a, b, a + V, b + V])           # [1024]
        xi[:, k * 64:(k + 1) * 64] = _wrap16(idx)
    per["xidx"] = xi
    qt = np.zeros((P, NQB), np.int16)
    for g in range(NQ):
        for rb in range(B // P):
            qt[:, g * 2 + rb] = wid_loc[rb * P:(rb + 1) * P, g].astype(np.int16)
    per["qtidx"] = qt
    tvq = np.zeros((P, B2), f32)
    tvq[:L, 0:B] = tv_loc.T
    tvq[:L, B:B2] = tv_loc.T
    tvq[L, :] = 1.0
    per["tvq2"] = tvq
    return per


def kernel(**inputs):
    key = "k"
    if key not in _RUNNER:
        nc = build_nc(reps=1)
        _RUNNER[key] = _BassRunner(nc, NCORES)
    runner = _RUNNER[key]
    shared = _prep_shared(inputs)
    in_maps = []
    for c in range(NCORES):
        m = dict(shared)
        m.update(_prep_core(inputs, c))
        in_maps.append(m)
    res = runner(in_maps)
    qls = sum(float(r["out"][0, 0]) for r in res)
    qcnt = sum(float(r["out"][0, 1]) for r in res)
    pls = sum(float(r["out"][0, 2]) for r in res)
    psgn = sum(float(r["out"][0, 3]) for r in res)
    q_loss = np.float32(qls / B_FULL)
    p_loss = np.float32(pls / B_FULL)
    q_acc = np.float32(qcnt / (NQ * B_FULL))
    p_acc = np.float32((NCORES * 24 * B - psgn) / (NP * B_FULL))
    return q_loss, p_loss, q_acc, p_acc
